# revision 22
# baseline (speedup 1.0000x reference)
"""Trainium2 Bass kernel for nn_Block_4440996184125 (dense_cnn).

Sharding: pure data parallelism over batch B=8 -> one batch item per
NeuronCore. Channel-major layout [C on partitions, W on free axis].

v2 design notes (vs v1 baseline):
  - Per-position softmax/normalize statistics accumulate into PSUM tiles
    PACKED along partitions (chunk ci -> rows 16*ci), so sqrt/recip/exp
    run as a handful of [128, 512] batched ops per half.
  - reciprocal_approx_fast (custom DVE op, ~5x faster) replaces all
    reciprocals; Act does Sqrt/Exp/Silu (Copy/Square live in every act
    table -> no table reloads from them).
  - Elementwise moved off Pool (0.42x efficiency) onto DVE (bf16 2x
    mode) and Act.
  - x read from HBM once; bf16 copy (xb) kept in SBUF for balance.
  - Loops software-pipelined so the PE queue is continuously fed
    (conv/res0 matmuls of chunk s+1 are emitted before the stat/res1
    matmuls of chunk s that wait on elementwise results).
"""
import sys
sys.path.insert(0, "/opt/trn_rl_repo")

import numpy as np
import ml_dtypes

import concourse.bass as bass
import concourse.bacc as bacc
import concourse.tile as tile
from concourse import mybir
from concourse.bass_utils import run_bass_kernel_spmd

F32 = mybir.dt.float32
F32R = mybir.dt.float32r
BF16 = mybir.dt.bfloat16
AF = mybir.ActivationFunctionType
OP = mybir.AluOpType

EPS = 1e-4
OFFSET = -2.0
MIN_BAL = 0.01
CLIP = 256.0
bf = ml_dtypes.bfloat16


# ----------------------------------------------------------------------------
# host-side prep
# ----------------------------------------------------------------------------

def _wnorm(w, gain=1.0):
    O = w.shape[0]
    fan_in = int(np.prod(w.shape[1:]))
    wf = w.reshape(O, -1).astype(np.float64)
    n = np.sqrt((wf ** 2).sum(1, keepdims=True))
    wn = wf / (EPS + n / np.sqrt(fan_in)) * (gain / np.sqrt(fan_in))
    return wn.astype(np.float32)


def _gconv1(emb, wn, groups):
    O, ig = wn.shape
    og = O // groups
    out = np.empty(O, np.float32)
    for g in range(groups):
        out[g * og:(g + 1) * og] = wn[g * og:(g + 1) * og] @ emb[g * ig:(g + 1) * ig]
    return out


def _host_prep(inputs, b):
    emb = np.asarray(inputs['emb'])[b, :, 0, 0].astype(np.float32)
    c = _gconv1(emb, _wnorm(np.asarray(inputs['w_qkv_emb']),
                            float(inputs['gain_qkv'])), 4) + 1.0
    c2 = _gconv1(emb, _wnorm(np.asarray(inputs['w_emb']),
                             float(inputs['emb_gain'])), 4) + 1.0

    def bal(wb, gain):
        logits = _gconv1(emb, _wnorm(np.asarray(wb), gain), 1) + OFFSET
        t = np.clip(1 / (1 + np.exp(-logits)), MIN_BAL, 1 - MIN_BAL)
        r = 1.0 / np.sqrt((1 - t) ** 2 + t ** 2)
        return ((1 - t) * r).astype(np.float32), (t * r).astype(np.float32)

    a1, b1 = bal(inputs['w_bal_attn'], float(inputs['gain_bal_attn']))
    a2, b2 = bal(inputs['w_bal_res'], float(inputs['gain_bal_res']))

    wq = _wnorm(np.asarray(inputs['w_q']))
    wk = _wnorm(np.asarray(inputs['w_k']))
    wv = _wnorm(np.asarray(inputs['w_v']))
    # fold balance gate b1 AND the x8 from v's normalize into w_proj
    wp = _wnorm(np.asarray(inputs['w_proj'])).copy()
    for g in range(4):
        wp[g * 64:(g + 1) * 64] *= b1[g] * 8.0

    def fold_c(w):
        wf = w.copy()
        for g in range(4):
            wf[g * 64:(g + 1) * 64] *= c[g * 64:(g + 1) * 64][None, :]
        return wf
    wqf, wkf, wvf = fold_c(wq), fold_c(wk), fold_c(wv)

    w0 = (_wnorm(np.asarray(inputs['w_res0'])) * c2[:, None]).reshape(1024, 64, 3)
    w1 = _wnorm(np.asarray(inputs['w_res1'])).reshape(256, 256, 3).copy()
    for g in range(4):
        w1[g * 64:(g + 1) * 64] *= b2[g] / 0.596

    cons = {}

    def blockdiag(w):
        out = np.zeros((2, 128, 128), np.float32)
        for t in range(2):
            for d in range(2):
                g = 2 * t + d
                out[t, 64 * d:64 * d + 64, 64 * d:64 * d + 64] = \
                    w[g * 64:(g + 1) * 64].T
        return out

    cons['lq'] = blockdiag(wqf)
    cons['lk'] = blockdiag(wkf)
    cons['lv'] = blockdiag(wvf)
    cons['lp'] = blockdiag(wp).astype(bf)

    # res0 1x3 grouped conv, K-packed: l0a = taps 0,1 (K=128), l0b = tap 2
    l0a = np.zeros((4, 2, 128, 128), np.float32)
    l0b = np.zeros((4, 2, 64, 128), np.float32)
    for g in range(4):
        for j in range(2):
            rows = slice(256 * g + 128 * j, 256 * g + 128 * j + 128)
            l0a[g, j, 0:64] = w0[rows, :, 0].T
            l0a[g, j, 64:128] = w0[rows, :, 1].T
            l0b[g, j] = w0[rows, :, 2].T
    cons['l0a'] = l0a.astype(bf)
    cons['l0b'] = l0b.astype(bf)

    # res1: per (t, s*3+tap, col): [128 K, 64 M]
    l1 = np.zeros((2, 6, 2, 128, 64), np.float32)
    for t in range(2):
        for s in range(2):
            for tap in range(3):
                for col in range(2):
                    orow = slice(128 * t + 64 * col, 128 * t + 64 * col + 64)
                    l1[t, s * 3 + tap, col] = \
                        w1[orow, 128 * s:128 * s + 128, tap].T
    cons['l1'] = l1.astype(bf)

    # selectors for TA (norm) reductions -> 16 (g,h)-pair rows
    rnq = np.zeros((2, 128, 16), np.float32)
    rnk = np.zeros((2, 128, 16), np.float32)
    for t in range(2):
        for kk in range(128):
            gr = kk // 64 + 2 * t
            for g in range(4):
                for h in range(4):
                    if g == gr:
                        rnq[t, kk, 4 * g + h] = 1.0
                    if h == gr:
                        rnk[t, kk, 4 * g + h] = 1.0
    cons['rnq'] = rnq.astype(bf)
    cons['rnk'] = rnk.astype(bf)

    rg = np.zeros((4, 2, 128, 16), np.float32)
    for r in range(4):
        for t in range(2):
            for j in range(2):
                g = 2 * t + j
                rg[r, t, 64 * j:64 * j + 64, 4 * g + ((g + r) % 4)] = 1.0
    cons['rG'] = rg.astype(bf)

    rz = np.zeros((16, 16), np.float32)
    for gp in range(4):
        for hp in range(4):
            for h in range(4):
                rz[4 * gp + hp, 4 * gp + h] = 1.0
    # replicated per row-position (stored at partition 32*i)
    cons['rZ4'] = np.broadcast_to(rz, (3, 16, 16)).astype(bf)

    bsel = np.zeros((4, 2, 16, 128), np.float32)
    for r in range(4):
        for t in range(2):
            for m in range(128):
                g = 2 * t + m // 64
                bsel[r, t, 4 * g + ((g + r) % 4), m] = 1.0
    cons['bsel4'] = np.broadcast_to(bsel, (3, 4, 2, 16, 128)).astype(bf)

    # ibsel4[t,d]: [4,128] selector broadcasting invc row 2t+d
    ibsel4 = np.zeros((2, 2, 4, 128), np.float32)
    for t in range(2):
        for d in range(2):
            ibsel4[t, d, 2 * t + d, :] = 1.0
    cons['ibsel4'] = ibsel4.astype(bf)

    # onesg[g]: [128,4] column-g ones (nsp accumulation selector)
    onesg = np.zeros((4, 128, 4), np.float32)
    for g in range(4):
        onesg[g, :, g] = 1.0
    cons['onesg'] = onesg.astype(bf)

    def pvec(vals):
        out = np.empty((2, 128, 1), np.float32)
        for t in range(2):
            for p in range(128):
                out[t, p, 0] = vals[2 * t + p // 64]
        return out
    cons['a1v'] = pvec(a1)
    cons['a2v'] = pvec(a2)
    return cons


CONST_SPECS = [
    ('lq', (2, 128, 128), F32R), ('lk', (2, 128, 128), F32R),
    ('lv', (2, 128, 128), F32R),
    ('lp', (2, 128, 128), BF16),
    ('l0a', (4, 2, 128, 128), BF16), ('l0b', (4, 2, 64, 128), BF16),
    ('l1', (2, 6, 2, 128, 64), BF16),
    ('rnq', (2, 128, 16), BF16), ('rnk', (2, 128, 16), BF16),
    ('rG', (4, 2, 128, 16), BF16),
    ('rZ4', (3, 16, 16), BF16), ('bsel4', (3, 4, 2, 16, 128), BF16),
    ('ibsel4', (2, 2, 4, 128), BF16), ('onesg', (4, 128, 4), BF16),
    ('a1v', (2, 128, 1), F32), ('a2v', (2, 128, 1), F32),
]
_SHAPES = {n: s for n, s, _ in CONST_SPECS}


def _const_r0(name, i):
    """Partition offset of const tile i within the [128, F] mega tensor."""
    if name == 'l0b':
        return 64
    if name == 'rZ4':
        return 32 * i            # i == row position
    if name == 'bsel4':
        return 32 * (i // 8)     # i == ravel(rowpos, r, t)
    return 0


def _mega_layout():
    """Pack all consts into 3 dtype-major mega tensors [128, F]."""
    offs = {}
    sizes = {'r': 0, 'b': 0, 'f': 0}
    key = {F32R: 'r', BF16: 'b', F32: 'f'}
    for name, shape, dt in CONST_SPECS:
        k = key[dt]
        ntile = int(np.prod(shape[:-2])) if len(shape) > 2 else 1
        F = shape[-1]
        for i in range(ntile):
            offs[(name, i)] = (k, sizes[k])
            sizes[k] += F
    return offs, sizes


_MEGA_OFFS, _MEGA_SIZES = _mega_layout()


def _pack_consts(cons):
    arrs = {'r': np.zeros((128, _MEGA_SIZES['r']), np.float32),
            'b': np.zeros((128, _MEGA_SIZES['b']), bf),
            'f': np.zeros((128, _MEGA_SIZES['f']), np.float32)}
    for name, shape, dt in CONST_SPECS:
        ntile = int(np.prod(shape[:-2])) if len(shape) > 2 else 1
        P, F = shape[-2], shape[-1]
        flat = np.asarray(cons[name]).reshape(ntile, P, F)
        for i in range(ntile):
            k, off = _MEGA_OFFS[(name, i)]
            r0 = _const_r0(name, i)
            arrs[k][r0:r0 + P, off:off + F] = flat[i]
    return {'cr': arrs['r'], 'cb': arrs['b'], 'cf': arrs['f']}


# v-plane index for rotation (r, t) given VL plane order [v_t0, v_t1, rot1,
# rot3]
_VMAP = {0: 0, 1: 2, 2: 1, 3: 3}


# ----------------------------------------------------------------------------
# bass builder
# ----------------------------------------------------------------------------

def build(W, n_cores=8, reps=1, variant=''):
    nc = bacc.Bacc("TRN2", target_bir_lowering=False, debug=False,
                   num_devices=n_cores)
    xin = nc.declare_dram_parameter("x", [2, 128, W], F32R, isOutput=False)
    dcr = nc.declare_dram_parameter("cr", [128, _MEGA_SIZES['r']], F32R,
                                    isOutput=False)
    dcb = nc.declare_dram_parameter("cb", [128, _MEGA_SIZES['b']], BF16,
                                    isOutput=False)
    dcf = nc.declare_dram_parameter("cf", [128, _MEGA_SIZES['f']], F32,
                                    isOutput=False)
    xout = nc.declare_dram_parameter("out", [2, 128, W], F32, isOutput=True)

    NA = 512
    W2 = W // 2
    assert W2 % NA == 0
    nah = W2 // NA          # A-chunks per half (8 -> stat rows 16*ci)
    assert nah == 8
    CB = 510
    hchunks = []            # B-chunks within one half (local coords)
    c0 = 0
    while c0 < W2:
        hchunks.append((c0, min(CB, W2 - c0)))
        c0 += CB
    nbh = len(hchunks)
    assert 4 * nbh <= 128

    with tile.TileContext(nc) as tc:
        with (
            tc.tile_pool(name="wpool", bufs=1) as wp,
            tc.tile_pool(name="xres", bufs=1) as xrp,
        ):
            mega = {}
            for mk, dp, dt in (('r', dcr, F32R), ('b', dcb, BF16),
                               ('f', dcf, F32)):
                mt = wp.tile([128, _MEGA_SIZES[mk]], dt, tag=f"mega_{mk}",
                             name=f"mega_{mk}")
                nc.gpsimd.dma_start(mt[:], dp[:])
                mega[mk] = mt

            def cget(name, *idx):
                shape = _SHAPES[name]
                i = (int(np.ravel_multi_index(idx, shape[:-2]))
                     if len(shape) > 2 else 0)
                k, off = _MEGA_OFFS[(name, i)]
                P, F = shape[-2], shape[-1]
                r0 = _const_r0(name, i)
                return mega[k][r0:r0 + P, off:off + F]

            # persistent row state: attention output + 2-col zero pad, bf16
            xres = [xrp.tile([128, W + 4], BF16, tag=f"xres{t}",
                             name=f"xres{t}") for t in range(2)]
            for t in range(2):
                nc.vector.memset(xres[t][:, 0:2], 0.0)
                nc.vector.memset(xres[t][:, W + 2:W + 4], 0.0)

            for _rep in range(reps):
                do_A = variant != 'noA'
                do_B = variant != 'noB'
                # ============ SWEEP A (attention -> xres), per half ========
                for h in range(2 if do_A else 0):
                    H0 = h * W2
                    with tc.tile_pool(name="arow", bufs=1) as arp:
                        # v planes: 0 = v_t0, 1 = v_t1, 2/3 = rotations
                        VL = arp.tile([128, 4, W2], BF16, tag="VL", name="VL")
                        xb = arp.tile([128, 2, W2], BF16, tag="xb", name="xb")
                        # packed stats: chunk ci -> rows 32*(ci%3)+0:16,
                        # col block (ci//3)*NA
                        NP = 3 * NA
                        Ep = arp.tile([128, NP], BF16, tag="Ep", name="Ep")
                        ap16 = arp.tile([128, NP], BF16, tag="ap16",
                                        name="ap16")
                        TQp = arp.tile([128, NP], F32, tag="TQp", name="TQp")
                        TKp = arp.tile([128, NP], F32, tag="TKp", name="TKp")
                        TVp = arp.tile([128, NP], F32, tag="TVp", name="TVp")
                        TBs = arp.tile([128, NP], BF16, tag="TBs", name="TBs")
                        Zs = arp.tile([128, NP], F32, tag="Zs", name="Zs")

                        def pk(ci):
                            """(row slice, col slice) of chunk ci in packed"""
                            rp, cb = ci % 3, (ci // 3) * NA
                            return (slice(32 * rp, 32 * rp + 16),
                                    slice(cb, cb + NA))

                        with (
                            tc.tile_pool(name="axin", bufs=3) as axin,
                            tc.tile_pool(name="acv", bufs=2,
                                         space="PSUM") as acv,
                            tc.tile_pool(name="ast", bufs=2,
                                         space="PSUM") as ast,
                            tc.tile_pool(name="asb", bufs=2) as asb,
                        ):
                            # software pipeline: stage1(s) = dma+conv+
                            # elementwise; stage2(s) = stat matmuls + pack
                            pend = {}

                            def a1_stage1(ci):
                                cl = ci * NA
                                cg = H0 + cl
                                xa = axin.tile([128, 2, NA], F32R,
                                               tag="xa", name="xa")
                                for t in range(2):
                                    nc.gpsimd.dma_start(
                                        xa[:, t, :],
                                        xin[t][:, cg:cg + NA])
                                nc.scalar.activation(
                                    xb[:, :, cl:cl + NA],
                                    xa[:].bitcast(F32), AF.Copy)

                                for lname, pn in (('lq', 'q'), ('lk', 'k'),
                                                  ('lv', 'v')):
                                    p = acv.tile([128, 2, NA], F32,
                                                 tag="cv", name="cv")
                                    for t in range(2):
                                        nc.tensor.matmul(
                                            p[:, t, :], cget(lname, t)[:],
                                            xa[:, t, :],
                                            start=True, stop=True)
                                    # drain PSUM quickly
                                    if pn == 'q':
                                        qs = asb.tile([128, 2, NA], BF16,
                                                      tag="qs", name="qs")
                                        nc.scalar.activation(qs[:], p[:],
                                                             AF.Copy)
                                    elif pn == 'k':
                                        ks = asb.tile([128, 2, NA], BF16,
                                                      tag="ks", name="ks")
                                        nc.scalar.activation(ks[:], p[:],
                                                             AF.Copy)
                                    else:
                                        nc.vector.tensor_copy(
                                            VL[:, 0:2, cl:cl + NA], p[:])

                                # k rotated one group: partition-move DMA
                                kr = asb.tile([128, 2, NA], BF16,
                                              tag="kr", name="kr")
                                nc.gpsimd.dma_start(kr[0:64, 0, :],
                                                    ks[64:128, 0, :])
                                nc.gpsimd.dma_start(kr[64:128, 0, :],
                                                    ks[0:64, 1, :])
                                nc.gpsimd.dma_start(kr[0:64, 1, :],
                                                    ks[64:128, 1, :])
                                nc.gpsimd.dma_start(kr[64:128, 1, :],
                                                    ks[0:64, 0, :])

                                sqq = asb.tile([128, 2, NA], BF16,
                                               tag="sqq", name="sqq")
                                sqk = asb.tile([128, 2, NA], BF16,
                                               tag="sqk", name="sqk")
                                sqv = asb.tile([128, 2, NA], BF16,
                                               tag="sqv", name="sqv")
                                nc.vector.tensor_tensor(
                                    out=sqq[:], in0=qs[:], in1=qs[:],
                                    op=OP.mult)
                                nc.gpsimd.tensor_tensor(
                                    out=sqk[:], in0=ks[:], in1=ks[:],
                                    op=OP.mult)
                                nc.scalar.activation(
                                    sqv[:], VL[:, 0:2, cl:cl + NA],
                                    AF.Square)

                                pg = asb.tile([128, 4, 2, NA], BF16,
                                              tag="pg", name="pg")
                                nc.vector.tensor_tensor(
                                    out=pg[:, 0, :, :], in0=qs[:],
                                    in1=ks[:], op=OP.mult)
                                nc.vector.tensor_tensor(
                                    out=pg[:, 1, :, :], in0=qs[:],
                                    in1=kr[:], op=OP.mult)
                                for t in range(2):
                                    eng = nc.gpsimd if t == 0 \
                                        else nc.vector
                                    eng.tensor_tensor(
                                        out=pg[:, 2, t, :],
                                        in0=qs[:, t, :],
                                        in1=ks[:, 1 - t, :], op=OP.mult)
                                    nc.vector.tensor_tensor(
                                        out=pg[:, 3, t, :],
                                        in0=qs[:, t, :],
                                        in1=kr[:, 1 - t, :], op=OP.mult)
                                pend[ci] = (sqq, sqk, sqv, pg)

                            def a1_stage2(ci):
                                sqq, sqk, sqv, pg = pend.pop(ci)
                                # q/k/v norms at quadrants of one PSUM tile
                                TS = ast.tile([128, NA], F32, tag="TS",
                                              name="TS")
                                TBt = ast.tile([16, NA], F32, tag="TBt",
                                               name="TBt")
                                for t in range(2):
                                    nc.tensor.matmul(
                                        TS[0:16, :], cget('rnq', t)[:],
                                        sqq[:, t, :], start=(t == 0),
                                        stop=(t == 1))
                                    nc.tensor.matmul(
                                        TS[32:48, :], cget('rnk', t)[:],
                                        sqk[:, t, :], start=(t == 0),
                                        stop=(t == 1))
                                    nc.tensor.matmul(
                                        TS[64:80, :], cget('rnk', t)[:],
                                        sqv[:, t, :], start=(t == 0),
                                        stop=(t == 1))
                                n = 0
                                for r in range(4):
                                    for t in range(2):
                                        nc.tensor.matmul(
                                            TBt[:],
                                            cget('rG', r, t)[:],
                                            pg[:, r, t, :],
                                            start=(n == 0), stop=(n == 7))
                                        n += 1
                                # pack into SBUF stat buffers
                                rs, cs = pk(ci)
                                nc.vector.tensor_copy(TQp[rs, cs],
                                                      TS[0:16, :])
                                nc.vector.tensor_copy(TKp[rs, cs],
                                                      TS[32:48, :])
                                nc.scalar.activation(TVp[rs, cs],
                                                     TS[64:80, :], AF.Copy)
                                nc.vector.tensor_copy(TBs[rs, cs],
                                                      TBt[:])

                            for s in range(nah + 1):
                                if s < nah:
                                    a1_stage1(s)
                                if s >= 1:
                                    a1_stage2(s - 1)

                        # ---- between-pass batched softmax rows ----
                        nc.gpsimd.dma_start(VL[0:64, 2, :],
                                            VL[64:128, 0, :])
                        nc.gpsimd.dma_start(VL[64:128, 2, :],
                                            VL[0:64, 1, :])
                        nc.gpsimd.dma_start(VL[0:64, 3, :],
                                            VL[64:128, 1, :])
                        nc.gpsimd.dma_start(VL[64:128, 3, :],
                                            VL[0:64, 0, :])

                        # tile reuse: s1->TQp, iqk->TKp, sv->P, iv->TVp,
                        # izr->P, izv->TKp
                        P = arp.tile([128, NP], F32, tag="P", name="P")
                        mrow = arp.tile([128, NP], BF16, tag="mrow",
                                        name="mrow")
                        nc.vector.tensor_tensor(out=P[:], in0=TQp[:],
                                                in1=TKp[:], op=OP.mult)
                        nc.scalar.activation(TQp[:], P[:], AF.Sqrt)
                        nc.vector.reciprocal_approx_fast(out=TKp[:],
                                                         in_=TQp[:])
                        nc.vector.tensor_tensor(out=mrow[:], in0=TBs[:],
                                                in1=TKp[:], op=OP.mult)
                        nc.scalar.activation(Ep[:], mrow[:], AF.Exp,
                                             scale=8.0)
                        nc.scalar.activation(P[:], TVp[:], AF.Sqrt)
                        nc.vector.reciprocal_approx_fast(out=TVp[:],
                                                         in_=P[:])

                        # softmax denominators
                        with tc.tile_pool(name="azp", bufs=2,
                                          space="PSUM") as azp:
                            for ci in range(nah):
                                rs, cs = pk(ci)
                                rp_ = ci % 3
                                Zp = azp.tile([16, NA], F32, tag="Zp",
                                              name="Zp")
                                nc.tensor.matmul(Zp[:], cget('rZ4', rp_)[:],
                                                 Ep[rs, cs],
                                                 start=True, stop=True)
                                nc.vector.tensor_copy(Zs[rs, cs], Zp[:])
                            nc.vector.reciprocal_approx_fast(out=P[:],
                                                             in_=Zs[:])
                            nc.vector.tensor_tensor(out=TKp[:], in0=P[:],
                                                    in1=TVp[:], op=OP.mult)
                            nc.vector.tensor_tensor(out=ap16[:], in0=Ep[:],
                                                    in1=TKp[:], op=OP.mult)

                        # ---- pass 2: attention apply + proj + balance ----
                        with (
                            tc.tile_pool(name="pdbc", bufs=4,
                                         space="PSUM") as pdbc,
                            tc.tile_pool(name="pprj", bufs=2,
                                         space="PSUM") as pprj,
                            tc.tile_pool(name="tsb", bufs=3) as tsb,
                        ):
                            for ci in range(nah):
                                cl = ci * NA
                                cg = H0 + cl
                                rs, cs = pk(ci)
                                rp_ = ci % 3
                                for t in range(2):
                                    pp = pprj.tile([128, NA], F32, tag="prj",
                                                   name="prj")
                                    dbcs = []
                                    for r in range(4):
                                        dbc = pdbc.tile([128, NA], F32,
                                                        tag="dbc", name="dbc")
                                        nc.tensor.matmul(
                                            dbc[:],
                                            cget('bsel4', rp_, r, t)[:],
                                            ap16[rs, cs], start=True,
                                            stop=True)
                                        dbcs.append(dbc)
                                    prs = []
                                    for r in range(4):
                                        vsl = VL[:, _VMAP[(r + 2 * t) % 4],
                                                 cl:cl + NA]
                                        pr = tsb.tile([128, NA], BF16,
                                                      tag=f"pr{r}",
                                                      name=f"pr{r}")
                                        if r == 2:
                                            nc.vector.tensor_tensor(
                                                out=pr[:], in0=dbcs[r][:],
                                                in1=vsl, op=OP.mult)
                                        else:
                                            db = tsb.tile([128, NA], BF16,
                                                          tag=f"dbb{r}",
                                                          name=f"dbb{r}")
                                            nc.scalar.activation(db[:],
                                                                 dbcs[r][:],
                                                                 AF.Copy)
                                            eng = nc.gpsimd if r == 3 \
                                                else nc.vector
                                            eng.tensor_tensor(
                                                out=pr[:], in0=db[:],
                                                in1=vsl, op=OP.mult)
                                        prs.append(pr)
                                    for r in range(4):
                                        nc.tensor.matmul(pp[:],
                                                         cget('lp', t)[:],
                                                         prs[r][:],
                                                         start=(r == 0),
                                                         stop=(r == 3))
                                    nc.vector.scalar_tensor_tensor(
                                        out=xres[t][:, 2 + cg:2 + cg + NA],
                                        in0=xb[:, t, cl:cl + NA],
                                        scalar=cget('a1v', t)[:],
                                        in1=pp[:], op0=OP.mult, op1=OP.add)

                # fallbacks so every variant still writes the full output
                if not do_A:
                    with tc.tile_pool(name="fxin", bufs=3) as fxi:
                        for ci in range(W // NA):
                            cg = ci * NA
                            for t in range(2):
                                xa = fxi.tile([128, NA], F32, tag=f"fx{t}",
                                              name=f"fx{t}")
                                nc.sync.dma_start(
                                    xa[:], xin[t][:, cg:cg + NA].bitcast(F32))
                                nc.vector.tensor_copy(
                                    xres[t][:, 2 + cg:2 + cg + NA], xa[:])
                if not do_B:
                    with tc.tile_pool(name="fob", bufs=3) as fob:
                        for ci in range(W // NA):
                            cg = ci * NA
                            for t in range(2):
                                xo = fob.tile([128, NA], F32, tag=f"fo{t}",
                                              name=f"fo{t}")
                                nc.vector.tensor_copy(
                                    xo[:], xres[t][:, 2 + cg:2 + cg + NA])
                                nc.sync.dma_start(xout[t][:, cg:cg + NA],
                                                  xo[:])
                # ============ SWEEP B (res block -> out), per half =========
                for h in range(2 if do_B else 0):
                    H0 = h * W2
                    with tc.tile_pool(name="brow", bufs=1) as brp:
                        # y2 tiles: per (t,d): [128, 2(j), W2+2]
                        y2r = {}
                        for t in range(2):
                            for d in range(2):
                                y2r[(t, d)] = brp.tile(
                                    [128, 2, W2 + 2], BF16,
                                    tag=f"y2r{t}{d}", name=f"y2r{t}{d}")
                        # chunk ci -> rows 32*(ci%4)+0:4, col (ci//4)*NA
                        NB3 = ((nbh + 3) // 4) * NA
                        nsPk = brp.tile([128, NB3], F32, tag="nsPk",
                                        name="nsPk")
                        invp = brp.tile([128, NB3], F32, tag="invp",
                                        name="invp")

                        def pkb(ci):
                            rp_, cb = ci % 4, (ci // 4) * NA
                            return (slice(32 * rp_, 32 * rp_ + 4),
                                    slice(cb, cb + NA))

                        with (
                            tc.tile_pool(name="bst", bufs=3) as bst,
                            tc.tile_pool(name="by2", bufs=4,
                                         space="PSUM") as by2,
                            tc.tile_pool(name="bns", bufs=2,
                                         space="PSUM") as bns,
                            tc.tile_pool(name="bsq", bufs=3) as bsq,
                        ):
                            pend = {}

                            def b1_stage1(ci):
                                cl, C = hchunks[ci]
                                cg = H0 + cl
                                NW = C + 2
                                sqys = []
                                for t in range(2):
                                    for d in range(2):
                                        S = bst.tile([128, NW + 1], BF16,
                                                     tag=f"S{d}",
                                                     name=f"S{d}")
                                        src = xres[t][64 * d:
                                                      64 * d + 64, :]
                                        nc.sync.dma_start(
                                            S[0:64, :],
                                            src[:, cg:cg + NW + 1])
                                        nc.sync.dma_start(
                                            S[64:128, :],
                                            src[:, cg + 1:cg + NW + 2])
                                        g = 2 * t + d
                                        yb = y2r[(t, d)]
                                        sqy = bsq.tile(
                                            [128, 2, NW], BF16,
                                            tag=f"sqy{t}{d}",
                                            name=f"sqy{t}{d}")
                                        for j in range(2):
                                            p = by2.tile([128, NW], F32,
                                                         tag="y2",
                                                         name="y2")
                                            nc.tensor.matmul(
                                                p[:],
                                                cget('l0a', g, j)[:],
                                                S[:, 0:NW],
                                                start=True, stop=False)
                                            nc.tensor.matmul(
                                                p[:],
                                                cget('l0b', g, j)[:],
                                                S[64:128, 1:NW + 1],
                                                start=False, stop=True)
                                            ysl = yb[:, j, cl:cl + NW]
                                            if j == 0:
                                                nc.scalar.activation(
                                                    ysl, p[:], AF.Copy)
                                                nc.vector.tensor_tensor(
                                                    out=sqy[:, j, :],
                                                    in0=ysl, in1=ysl,
                                                    op=OP.mult)
                                            else:
                                                nc.vector.tensor_copy(
                                                    ysl, p[:])
                                                if d == 0:
                                                    nc.scalar.activation(
                                                        sqy[:, j, :],
                                                        ysl, AF.Square)
                                                else:
                                                    nc.gpsimd.\
                                                        tensor_tensor(
                                                            out=sqy[:, j,
                                                                    :],
                                                            in0=ysl,
                                                            in1=ysl,
                                                            op=OP.mult)
                                        sqys.append((g, NW, sqy))
                                pend[ci] = sqys

                            def b1_stage2(ci):
                                NWc = hchunks[ci][1] + 2
                                nspc = bns.tile([4, NWc], F32, tag="nsp",
                                                name="nsp")
                                n = 0
                                for (g, NW, sqy) in pend.pop(ci):
                                    for j in range(2):
                                        nc.tensor.matmul(
                                            nspc[:, 0:NW],
                                            cget('onesg', g)[:],
                                            sqy[:, j, :],
                                            start=(n == 0),
                                            stop=(n == 7))
                                        n += 1
                                rs, cs = pkb(ci)
                                nc.vector.tensor_copy(
                                    nsPk[rs, cs.start:cs.start + NWc],
                                    nspc[:])

                            for s in range(nbh + 1):
                                if s < nbh:
                                    b1_stage1(s)
                                if s >= 1:
                                    b1_stage2(s - 1)

                        # ---- between: batched inv norms ----
                        snp = brp.tile([128, NB3], F32, tag="snp",
                                       name="snp")
                        nc.scalar.activation(snp[:], nsPk[:], AF.Sqrt,
                                             scale=1.0 / 256)
                        nc.vector.reciprocal_approx_fast(out=invp[:],
                                                         in_=snp[:])

                        with (
                            tc.tile_pool(name="bibc", bufs=4,
                                         space="PSUM") as bibc,
                            tc.tile_pool(name="bres", bufs=2,
                                         space="PSUM") as bres,
                            tc.tile_pool(name="bsb", bufs=2) as bsb,
                        ):
                            pend2 = {}

                            def b2_stage1(ci):
                                cl, C = hchunks[ci]
                                cg = H0 + cl
                                NW = C + 2
                                # per-chunk inv-norm rows, bf16 (base 0)
                                invc = bsb.tile([4, NW], BF16, tag="invc",
                                                name="invc")
                                rsb, csb = pkb(ci)
                                nc.vector.tensor_copy(
                                    invc[:],
                                    invp[rsb, csb.start:csb.start + NW])
                                ucs = []
                                for t in range(2):
                                    zcu = bsb.tile([128, 4, NW], BF16,
                                                   tag=f"zc{t}",
                                                   name=f"zc{t}")
                                    uc = bsb.tile([128, 4, NW], BF16,
                                                  tag=f"uc{t}",
                                                  name=f"uc{t}")
                                    for d in range(2):
                                        ib = bibc.tile([128, NW], F32,
                                                       tag="ibc", name="ibc")
                                        nc.tensor.matmul(
                                            ib[:], cget('ibsel4', t, d)[:],
                                            invc[:],
                                            start=True, stop=True)
                                        ibb = bsb.tile([128, NW], BF16,
                                                       tag=f"ibcb{t}{d}",
                                                       name=f"ibcb{t}{d}")
                                        nc.vector.tensor_copy(ibb[:], ib[:])
                                        for j in range(2):
                                            eng = nc.vector if j == 0 \
                                                else nc.gpsimd
                                            eng.tensor_tensor(
                                                out=zcu[:, 2 * d + j, :],
                                                in0=y2r[(t, d)][:, j,
                                                                cl:cl + NW],
                                                in1=ibb[:], op=OP.mult)
                                    nc.scalar.activation(uc[:], zcu[:],
                                                         AF.Silu)
                                    if cg == 0:
                                        nc.vector.memset(uc[:, :, 0:1], 0.0)
                                    if cg + C == W:
                                        nc.vector.memset(
                                            uc[:, :, NW - 1:NW], 0.0)
                                    ucs.append(uc)
                                pend2[ci] = ucs

                            def b2_stage2(ci):
                                cl, C = hchunks[ci]
                                cg = H0 + cl
                                ucs = pend2.pop(ci)
                                for t in range(2):
                                    uc = ucs[t]
                                    rp = bres.tile([128, C], F32, tag="rp",
                                                   name="rp")
                                    for col in range(2):
                                        kidx = 0
                                        for s in range(2):
                                            for tap in range(3):
                                                nc.tensor.matmul(
                                                    rp[64 * col:
                                                       64 * col + 64, :],
                                                    cget('l1', t,
                                                         s * 3 + tap,
                                                         col)[:],
                                                    uc[:, 2 * col + s,
                                                       tap:tap + C],
                                                    start=(kidx == 0),
                                                    stop=(kidx == 5))
                                                kidx += 1
                                    xo = bsb.tile([128, C], F32,
                                                  tag=f"xo{t}",
                                                  name=f"xo{t}")
                                    nc.vector.scalar_tensor_tensor(
                                        out=xo[:],
                                        in0=xres[t][:, 2 + cg:2 + cg + C],
                                        scalar=cget('a2v', t)[:], in1=rp[:],
                                        op0=OP.mult, op1=OP.add)
                                    xc = bsb.tile([128, C], F32,
                                                  tag=f"xc{t}",
                                                  name=f"xc{t}")
                                    nc.gpsimd.tensor_scalar(
                                        out=xc[:], in0=xo[:], scalar1=CLIP,
                                        scalar2=-CLIP, op0=OP.min,
                                        op1=OP.max)
                                    nc.sync.dma_start(
                                        xout[t][:, cg:cg + C], xc[:])

                            for s in range(nbh + 1):
                                if s < nbh:
                                    b2_stage1(s)
                                if s >= 1:
                                    b2_stage2(s - 1)
    nc.compile()
    return nc


_BUILD_CACHE = {}


def _get_nc(W, reps=1, variant=''):
    key = (W, reps, variant)
    if key not in _BUILD_CACHE:
        _BUILD_CACHE[key] = build(W, reps=reps, variant=variant)
    return _BUILD_CACHE[key]


def make_in_map(inputs, b, W):
    x = np.asarray(inputs['x'], dtype=np.float32)
    cons = _host_prep(inputs, b)
    im = {'x': np.ascontiguousarray(x[b, :, 0, :].reshape(2, 128, W))}
    im.update(_pack_consts(cons))
    return im


def kernel(**inputs):
    x = np.asarray(inputs['x'], dtype=np.float32)
    B, C, H, W = x.shape
    nc = _get_nc(W)
    in_maps = [make_in_map(inputs, b, W) for b in range(B)]
    res = run_bass_kernel_spmd(nc, in_maps, list(range(B)))
    out = np.empty((B, C, H, W), np.float32)
    for b in range(B):
        out[b, :, 0, :] = np.asarray(res.results[b]['out'],
                                     dtype=np.float32).reshape(256, W)
    return out


# revision 31
# speedup vs baseline: 1.2872x; 1.2872x over previous
"""Trainium2 Bass kernel for nn_Block_4440996184125 (dense_cnn).

Sharding: pure data parallelism over batch B=8 -> one batch item per
NeuronCore. Channel-major layout [C on partitions, W on free axis].

v2 design notes (vs v1 baseline):
  - Per-position softmax/normalize statistics accumulate into PSUM tiles
    PACKED along partitions (chunk ci -> rows 16*ci), so sqrt/recip/exp
    run as a handful of [128, 512] batched ops per half.
  - reciprocal_approx_fast (custom DVE op, ~5x faster) replaces all
    reciprocals; Act does Sqrt/Exp/Silu (Copy/Square live in every act
    table -> no table reloads from them).
  - Elementwise moved off Pool (0.42x efficiency) onto DVE (bf16 2x
    mode) and Act.
  - x read from HBM once; bf16 copy (xb) kept in SBUF for balance.
  - Loops software-pipelined so the PE queue is continuously fed
    (conv/res0 matmuls of chunk s+1 are emitted before the stat/res1
    matmuls of chunk s that wait on elementwise results).
"""
import sys
sys.path.insert(0, "/opt/trn_rl_repo")

import numpy as np
import ml_dtypes

import concourse.bass as bass
import concourse.bacc as bacc
import concourse.tile as tile
from concourse import mybir
from concourse.bass_utils import run_bass_kernel_spmd

F32 = mybir.dt.float32
F32R = mybir.dt.float32r
BF16 = mybir.dt.bfloat16
AF = mybir.ActivationFunctionType
OP = mybir.AluOpType

EPS = 1e-4
OFFSET = -2.0
MIN_BAL = 0.01
CLIP = 256.0
bf = ml_dtypes.bfloat16


# ----------------------------------------------------------------------------
# host-side prep
# ----------------------------------------------------------------------------

def _wnorm(w, gain=1.0):
    O = w.shape[0]
    fan_in = int(np.prod(w.shape[1:]))
    wf = w.reshape(O, -1).astype(np.float64)
    n = np.sqrt((wf ** 2).sum(1, keepdims=True))
    wn = wf / (EPS + n / np.sqrt(fan_in)) * (gain / np.sqrt(fan_in))
    return wn.astype(np.float32)


def _gconv1(emb, wn, groups):
    O, ig = wn.shape
    og = O // groups
    out = np.empty(O, np.float32)
    for g in range(groups):
        out[g * og:(g + 1) * og] = wn[g * og:(g + 1) * og] @ emb[g * ig:(g + 1) * ig]
    return out


def _host_prep(inputs, b):
    emb = np.asarray(inputs['emb'])[b, :, 0, 0].astype(np.float32)
    c = _gconv1(emb, _wnorm(np.asarray(inputs['w_qkv_emb']),
                            float(inputs['gain_qkv'])), 4) + 1.0
    c2 = _gconv1(emb, _wnorm(np.asarray(inputs['w_emb']),
                             float(inputs['emb_gain'])), 4) + 1.0

    def bal(wb, gain):
        logits = _gconv1(emb, _wnorm(np.asarray(wb), gain), 1) + OFFSET
        t = np.clip(1 / (1 + np.exp(-logits)), MIN_BAL, 1 - MIN_BAL)
        r = 1.0 / np.sqrt((1 - t) ** 2 + t ** 2)
        return ((1 - t) * r).astype(np.float32), (t * r).astype(np.float32)

    a1, b1 = bal(inputs['w_bal_attn'], float(inputs['gain_bal_attn']))
    a2, b2 = bal(inputs['w_bal_res'], float(inputs['gain_bal_res']))

    wq = _wnorm(np.asarray(inputs['w_q']))
    wk = _wnorm(np.asarray(inputs['w_k']))
    wv = _wnorm(np.asarray(inputs['w_v']))
    # fold balance gate b1 AND the x8 from v's normalize into w_proj
    wp = _wnorm(np.asarray(inputs['w_proj'])).copy()
    for g in range(4):
        wp[g * 64:(g + 1) * 64] *= b1[g] * 8.0

    def fold_c(w):
        wf = w.copy()
        for g in range(4):
            wf[g * 64:(g + 1) * 64] *= c[g * 64:(g + 1) * 64][None, :]
        return wf
    wqf, wkf, wvf = fold_c(wq), fold_c(wk), fold_c(wv)

    w0 = (_wnorm(np.asarray(inputs['w_res0'])) * c2[:, None]).reshape(1024, 64, 3)
    w1 = _wnorm(np.asarray(inputs['w_res1'])).reshape(256, 256, 3).copy()
    for g in range(4):
        w1[g * 64:(g + 1) * 64] *= b2[g] / 0.596

    cons = {}

    def blockdiag(w):
        out = np.zeros((2, 128, 128), np.float32)
        for t in range(2):
            for d in range(2):
                g = 2 * t + d
                out[t, 64 * d:64 * d + 64, 64 * d:64 * d + 64] = \
                    w[g * 64:(g + 1) * 64].T
        return out

    cons['lq'] = blockdiag(wqf)
    cons['lk'] = blockdiag(wkf)
    cons['lv'] = blockdiag(wvf)
    cons['lp'] = blockdiag(wp).astype(bf)

    # res0 1x3 grouped conv, K-packed: l0a = taps 0,1 (K=128), l0b = tap 2
    l0a = np.zeros((4, 2, 128, 128), np.float32)
    l0b = np.zeros((4, 2, 64, 128), np.float32)
    for g in range(4):
        for j in range(2):
            rows = slice(256 * g + 128 * j, 256 * g + 128 * j + 128)
            l0a[g, j, 0:64] = w0[rows, :, 0].T
            l0a[g, j, 64:128] = w0[rows, :, 1].T
            l0b[g, j] = w0[rows, :, 2].T
    cons['l0a'] = l0a.astype(bf)
    cons['l0b'] = l0b.astype(bf)

    # res1: per (t, s*3+tap, col): [128 K, 64 M]
    l1 = np.zeros((2, 6, 2, 128, 64), np.float32)
    for t in range(2):
        for s in range(2):
            for tap in range(3):
                for col in range(2):
                    orow = slice(128 * t + 64 * col, 128 * t + 64 * col + 64)
                    l1[t, s * 3 + tap, col] = \
                        w1[orow, 128 * s:128 * s + 128, tap].T
    cons['l1'] = l1.astype(bf)

    # selectors for TA (norm) reductions -> 16 (g,h)-pair rows
    rnq = np.zeros((2, 128, 16), np.float32)
    rnk = np.zeros((2, 128, 16), np.float32)
    for t in range(2):
        for kk in range(128):
            gr = kk // 64 + 2 * t
            for g in range(4):
                for h in range(4):
                    if g == gr:
                        rnq[t, kk, 4 * g + h] = 1.0
                    if h == gr:
                        rnk[t, kk, 4 * g + h] = 1.0
    cons['rnq'] = rnq.astype(bf)
    cons['rnk'] = rnk.astype(bf)

    rg = np.zeros((4, 2, 128, 16), np.float32)
    for r in range(4):
        for t in range(2):
            for j in range(2):
                g = 2 * t + j
                rg[r, t, 64 * j:64 * j + 64, 4 * g + ((g + r) % 4)] = 1.0
    cons['rG'] = rg.astype(bf)

    rz = np.zeros((16, 16), np.float32)
    for gp in range(4):
        for hp in range(4):
            for h in range(4):
                rz[4 * gp + hp, 4 * gp + h] = 1.0
    # replicated per row-position (stored at partition 32*i)
    cons['rZ4'] = np.broadcast_to(rz, (3, 16, 16)).astype(bf)

    bsel = np.zeros((4, 2, 16, 128), np.float32)
    for r in range(4):
        for t in range(2):
            for m in range(128):
                g = 2 * t + m // 64
                bsel[r, t, 4 * g + ((g + r) % 4), m] = 1.0
    cons['bsel4'] = np.broadcast_to(bsel, (3, 4, 2, 16, 128)).astype(bf)

    # ibsel4[t,d]: [4,128] selector broadcasting invc row 2t+d
    ibsel4 = np.zeros((2, 2, 4, 128), np.float32)
    for t in range(2):
        for d in range(2):
            ibsel4[t, d, 2 * t + d, :] = 1.0
    cons['ibsel4'] = ibsel4.astype(bf)

    # onesg[g]: [128,4] column-g ones (nsp accumulation selector)
    onesg = np.zeros((4, 128, 4), np.float32)
    for g in range(4):
        onesg[g, :, g] = 1.0
    cons['onesg'] = onesg.astype(bf)

    def pdiag(vals):
        out = np.zeros((2, 128, 128), np.float32)
        for t in range(2):
            for p in range(128):
                out[t, p, p] = vals[2 * t + p // 64]
        return out
    cons['da1'] = pdiag(a1).astype(bf)
    cons['da2'] = pdiag(a2).astype(bf)
    return cons


CONST_SPECS = [
    ('lq', (2, 128, 128), F32R), ('lk', (2, 128, 128), F32R),
    ('lv', (2, 128, 128), F32R),
    ('lp', (2, 128, 128), BF16),
    ('l0a', (4, 2, 128, 128), BF16), ('l0b', (4, 2, 64, 128), BF16),
    ('l1', (2, 6, 2, 128, 64), BF16),
    ('rnq', (2, 128, 16), BF16), ('rnk', (2, 128, 16), BF16),
    ('rG', (4, 2, 128, 16), BF16),
    ('rZ4', (3, 16, 16), BF16), ('bsel4', (3, 4, 2, 16, 128), BF16),
    ('ibsel4', (2, 2, 4, 128), BF16), ('onesg', (4, 128, 4), BF16),
    ('da1', (2, 128, 128), BF16), ('da2', (2, 128, 128), BF16),
]
_SHAPES = {n: s for n, s, _ in CONST_SPECS}


def _const_r0(name, i):
    """Partition offset of const tile i within the [128, F] mega tensor."""
    if name == 'l0b':
        return 64
    if name == 'rZ4':
        return 32 * i            # i == row position
    if name == 'bsel4':
        return 32 * (i // 8)     # i == ravel(rowpos, r, t)
    return 0


def _mega_layout():
    """Pack all consts into 3 dtype-major mega tensors [128, F]."""
    offs = {}
    sizes = {'r': 0, 'b': 0, 'f': 0}
    key = {F32R: 'r', BF16: 'b', F32: 'f'}
    for name, shape, dt in CONST_SPECS:
        k = key[dt]
        ntile = int(np.prod(shape[:-2])) if len(shape) > 2 else 1
        F = shape[-1]
        for i in range(ntile):
            offs[(name, i)] = (k, sizes[k])
            sizes[k] += F
    return offs, sizes


_MEGA_OFFS, _MEGA_SIZES = _mega_layout()


def _pack_consts(cons):
    arrs = {'r': np.zeros((128, _MEGA_SIZES['r']), np.float32),
            'b': np.zeros((128, _MEGA_SIZES['b']), bf),
            'f': np.zeros((128, _MEGA_SIZES['f']), np.float32)}
    arrs = {k: v for k, v in arrs.items() if v.shape[1] > 0}
    for name, shape, dt in CONST_SPECS:
        ntile = int(np.prod(shape[:-2])) if len(shape) > 2 else 1
        P, F = shape[-2], shape[-1]
        flat = np.asarray(cons[name]).reshape(ntile, P, F)
        for i in range(ntile):
            k, off = _MEGA_OFFS[(name, i)]
            r0 = _const_r0(name, i)
            arrs[k][r0:r0 + P, off:off + F] = flat[i]
    out = {}
    for k, nm in (('r', 'cr'), ('b', 'cb'), ('f', 'cf')):
        if k in arrs:
            out[nm] = arrs[k]
    return out


# v-plane index for rotation (r, t) given VL plane order [v_t0, v_t1, rot1,
# rot3]
_VMAP = {0: 0, 1: 2, 2: 1, 3: 3}


# ----------------------------------------------------------------------------
# bass builder
# ----------------------------------------------------------------------------

def build(W, n_cores=8, reps=1, variant=''):
    nc = bacc.Bacc("TRN2", target_bir_lowering=False, debug=False,
                   num_devices=n_cores)
    xin = nc.declare_dram_parameter("x", [2, 128, W], F32R, isOutput=False)
    dmega = {}
    for nm, key, dt in (("cr", 'r', F32R), ("cb", 'b', BF16),
                        ("cf", 'f', F32)):
        if _MEGA_SIZES[key] > 0:
            dmega[key] = nc.declare_dram_parameter(
                nm, [128, _MEGA_SIZES[key]], dt, isOutput=False)
    xout = nc.declare_dram_parameter("out", [2, 128, W], F32, isOutput=True)

    NA = 512
    W2 = W // 2
    assert W2 % NA == 0
    nah = W2 // NA          # A-chunks per half (8 -> stat rows 16*ci)
    assert nah == 8
    CB = 510
    hchunks = []            # B-chunks within one half (local coords)
    c0 = 0
    while c0 < W2:
        hchunks.append((c0, min(CB, W2 - c0)))
        c0 += CB
    nbh = len(hchunks)
    assert 4 * nbh <= 128

    with tile.TileContext(nc) as tc:
        with (
            tc.tile_pool(name="wpool", bufs=1) as wp,
            tc.tile_pool(name="xres", bufs=1) as xrp,
        ):
            mega = {}
            for mk, dt in (('r', F32R), ('b', BF16), ('f', F32)):
                if mk not in dmega:
                    continue
                mt = wp.tile([128, _MEGA_SIZES[mk]], dt, tag=f"mega_{mk}",
                             name=f"mega_{mk}")
                nc.gpsimd.dma_start(mt[:], dmega[mk][:])
                mega[mk] = mt

            def cget(name, *idx):
                shape = _SHAPES[name]
                i = (int(np.ravel_multi_index(idx, shape[:-2]))
                     if len(shape) > 2 else 0)
                k, off = _MEGA_OFFS[(name, i)]
                P, F = shape[-2], shape[-1]
                r0 = _const_r0(name, i)
                return mega[k][r0:r0 + P, off:off + F]

            # persistent row state: attention output + 2-col zero pad, bf16
            xres = [xrp.tile([128, W + 4], BF16, tag=f"xres{t}",
                             name=f"xres{t}") for t in range(2)]
            for t in range(2):
                nc.vector.memset(xres[t][:, 0:2], 0.0)
                nc.vector.memset(xres[t][:, W + 2:W + 4], 0.0)

            for _rep in range(reps):
                do_A = variant != 'noA'
                do_B = variant != 'noB'
                # ============ SWEEP A (attention -> xres), per half ========
                def sweepA(h):
                    H0 = h * W2
                    with tc.tile_pool(name="arow", bufs=1) as arp:
                        # v planes: 0 = v_t0, 1 = v_t1, 2/3 = rotations
                        VL = arp.tile([128, 4, W2], BF16, tag="VL", name="VL")
                        xb = arp.tile([128, 2, W2], BF16, tag="xb", name="xb")
                        # packed stats: chunk ci -> rows 32*(ci%3)+0:16,
                        # col block (ci//3)*NA
                        NP = 3 * NA
                        Ep = arp.tile([128, NP], BF16, tag="Ep", name="Ep")
                        ap16 = arp.tile([128, NP], BF16, tag="ap16",
                                        name="ap16")
                        TQp = arp.tile([128, NP], F32, tag="TQp", name="TQp")
                        TKp = arp.tile([128, NP], F32, tag="TKp", name="TKp")
                        TVp = arp.tile([128, NP], F32, tag="TVp", name="TVp")
                        TBs = arp.tile([128, NP], BF16, tag="TBs", name="TBs")
                        Zs = arp.tile([128, NP], F32, tag="Zs", name="Zs")

                        def pk(ci):
                            """(row slice, col slice) of chunk ci in packed"""
                            rp, cb = ci % 3, (ci // 3) * NA
                            return (slice(32 * rp, 32 * rp + 16),
                                    slice(cb, cb + NA))

                        with (
                            tc.tile_pool(name="axin", bufs=3) as axin,
                            tc.tile_pool(name="acv", bufs=2,
                                         space="PSUM") as acv,
                            tc.tile_pool(name="ast", bufs=2,
                                         space="PSUM") as ast,
                            tc.tile_pool(name="asb", bufs=2) as asb,
                        ):
                            # software pipeline: stage1(s) = dma+conv+
                            # elementwise; stage2(s) = stat matmuls + pack
                            pend = {}

                            def a1_stage1(ci):
                                cl = ci * NA
                                cg = H0 + cl
                                xa = axin.tile([128, 2, NA], F32R,
                                               tag="xa", name="xa")
                                for t in range(2):
                                    nc.sync.dma_start(
                                        xa[:, t, :],
                                        xin[t][:, cg:cg + NA])
                                nc.scalar.activation(
                                    xb[:, :, cl:cl + NA],
                                    xa[:].bitcast(F32), AF.Copy)

                                for lname, pn in (('lq', 'q'), ('lk', 'k'),
                                                  ('lv', 'v')):
                                    p = acv.tile([128, 2, NA], F32,
                                                 tag="cv", name="cv")
                                    for t in range(2):
                                        nc.tensor.matmul(
                                            p[:, t, :], cget(lname, t)[:],
                                            xa[:, t, :],
                                            start=True, stop=True)
                                    # drain PSUM quickly
                                    if pn == 'q':
                                        qs = asb.tile([128, 2, NA], BF16,
                                                      tag="qs", name="qs")
                                        nc.scalar.activation(qs[:], p[:],
                                                             AF.Copy)
                                    elif pn == 'k':
                                        ks = asb.tile([128, 2, NA], BF16,
                                                      tag="ks", name="ks")
                                        nc.scalar.activation(ks[:], p[:],
                                                             AF.Copy)
                                    else:
                                        nc.vector.tensor_copy(
                                            VL[:, 0:2, cl:cl + NA], p[:])

                                # k rotated one group: partition-move DMA
                                kr = asb.tile([128, 2, NA], BF16,
                                              tag="kr", name="kr")
                                nc.sync.dma_start(kr[0:64, 0, :],
                                                  ks[64:128, 0, :])
                                nc.sync.dma_start(kr[64:128, 0, :],
                                                  ks[0:64, 1, :])
                                nc.sync.dma_start(kr[0:64, 1, :],
                                                  ks[64:128, 1, :])
                                nc.sync.dma_start(kr[64:128, 1, :],
                                                  ks[0:64, 0, :])

                                sqq = asb.tile([128, 2, NA], BF16,
                                               tag="sqq", name="sqq")
                                sqk = asb.tile([128, 2, NA], BF16,
                                               tag="sqk", name="sqk")
                                sqv = asb.tile([128, 2, NA], BF16,
                                               tag="sqv", name="sqv")
                                nc.vector.tensor_tensor(
                                    out=sqq[:], in0=qs[:], in1=qs[:],
                                    op=OP.mult)
                                nc.gpsimd.tensor_tensor(
                                    out=sqk[:], in0=ks[:], in1=ks[:],
                                    op=OP.mult)
                                nc.vector.tensor_tensor(
                                    out=sqv[:],
                                    in0=VL[:, 0:2, cl:cl + NA],
                                    in1=VL[:, 0:2, cl:cl + NA],
                                    op=OP.mult)

                                pg = asb.tile([128, 4, 2, NA], BF16,
                                              tag="pg", name="pg")
                                nc.vector.tensor_tensor(
                                    out=pg[:, 0, :, :], in0=qs[:],
                                    in1=ks[:], op=OP.mult)
                                nc.vector.tensor_tensor(
                                    out=pg[:, 1, :, :], in0=qs[:],
                                    in1=kr[:], op=OP.mult)
                                for t in range(2):
                                    eng = nc.gpsimd if t == 0 \
                                        else nc.vector
                                    eng.tensor_tensor(
                                        out=pg[:, 2, t, :],
                                        in0=qs[:, t, :],
                                        in1=ks[:, 1 - t, :], op=OP.mult)
                                    nc.vector.tensor_tensor(
                                        out=pg[:, 3, t, :],
                                        in0=qs[:, t, :],
                                        in1=kr[:, 1 - t, :], op=OP.mult)
                                pend[ci] = (sqq, sqk, sqv, pg)

                            def a1_stage2(ci):
                                sqq, sqk, sqv, pg = pend.pop(ci)
                                # q/k/v norms at quadrants of one PSUM tile
                                TS = ast.tile([128, NA], F32, tag="TS",
                                              name="TS")
                                TBt = ast.tile([16, NA], F32, tag="TBt",
                                               name="TBt")
                                for t in range(2):
                                    nc.tensor.matmul(
                                        TS[0:16, :], cget('rnq', t)[:],
                                        sqq[:, t, :], start=(t == 0),
                                        stop=(t == 1))
                                    nc.tensor.matmul(
                                        TS[32:48, :], cget('rnk', t)[:],
                                        sqk[:, t, :], start=(t == 0),
                                        stop=(t == 1))
                                    nc.tensor.matmul(
                                        TS[64:80, :], cget('rnk', t)[:],
                                        sqv[:, t, :], start=(t == 0),
                                        stop=(t == 1))
                                n = 0
                                for r in range(4):
                                    for t in range(2):
                                        nc.tensor.matmul(
                                            TBt[:],
                                            cget('rG', r, t)[:],
                                            pg[:, r, t, :],
                                            start=(n == 0), stop=(n == 7))
                                        n += 1
                                # pack into SBUF stat buffers
                                rs, cs = pk(ci)
                                nc.scalar.activation(TQp[rs, cs],
                                                     TS[0:16, :], AF.Copy)
                                nc.vector.tensor_copy(TKp[rs, cs],
                                                      TS[32:48, :])
                                nc.scalar.activation(TVp[rs, cs],
                                                     TS[64:80, :], AF.Copy)
                                nc.vector.tensor_copy(TBs[rs, cs],
                                                      TBt[:])

                            for s in range(nah + 1):
                                if s < nah:
                                    a1_stage1(s)
                                if s >= 1:
                                    a1_stage2(s - 1)

                        # ---- between-pass batched softmax rows ----
                        nc.gpsimd.dma_start(VL[0:64, 2, :],
                                            VL[64:128, 0, :])
                        nc.gpsimd.dma_start(VL[64:128, 2, :],
                                            VL[0:64, 1, :])
                        nc.gpsimd.dma_start(VL[0:64, 3, :],
                                            VL[64:128, 1, :])
                        nc.gpsimd.dma_start(VL[64:128, 3, :],
                                            VL[0:64, 0, :])

                        # tile reuse: s1->TQp, iqk->TKp, sv->P, iv->TVp,
                        # izr->P, izv->TKp
                        P = arp.tile([128, NP], F32, tag="P", name="P")
                        mrow = arp.tile([128, NP], BF16, tag="mrow",
                                        name="mrow")
                        nc.vector.tensor_tensor(out=P[:], in0=TQp[:],
                                                in1=TKp[:], op=OP.mult)
                        nc.scalar.activation(TQp[:], P[:], AF.Sqrt)
                        sv2 = arp.tile([128, NP], F32, tag="sv2", name="sv2")
                        nc.scalar.activation(sv2[:], TVp[:], AF.Sqrt)
                        nc.vector.reciprocal_approx_fast(out=TKp[:],
                                                         in_=TQp[:])
                        nc.vector.reciprocal_approx_fast(out=TVp[:],
                                                         in_=sv2[:])
                        nc.vector.tensor_tensor(out=mrow[:], in0=TBs[:],
                                                in1=TKp[:], op=OP.mult)
                        nc.scalar.activation(Ep[:], mrow[:], AF.Exp,
                                             scale=8.0)

                        # softmax denominators
                        with tc.tile_pool(name="azp", bufs=2,
                                          space="PSUM") as azp:
                            for ci in range(nah):
                                rs, cs = pk(ci)
                                rp_ = ci % 3
                                Zp = azp.tile([16, NA], F32, tag="Zp",
                                              name="Zp")
                                nc.tensor.matmul(Zp[:], cget('rZ4', rp_)[:],
                                                 Ep[rs, cs],
                                                 start=True, stop=True)
                                nc.scalar.activation(Zs[rs, cs], Zp[:],
                                                     AF.Copy)
                            nc.vector.reciprocal_approx_fast(out=P[:],
                                                             in_=Zs[:])
                            nc.vector.tensor_tensor(out=TKp[:], in0=P[:],
                                                    in1=TVp[:], op=OP.mult)
                            nc.vector.tensor_tensor(out=ap16[:], in0=Ep[:],
                                                    in1=TKp[:], op=OP.mult)

                        # ---- pass 2: attention apply + proj + balance ----
                        with (
                            tc.tile_pool(name="pdbc", bufs=4,
                                         space="PSUM") as pdbc,
                            tc.tile_pool(name="pprj", bufs=2,
                                         space="PSUM") as pprj,
                            tc.tile_pool(name="tsb", bufs=3) as tsb,
                        ):
                            for ci in range(nah):
                                cl = ci * NA
                                cg = H0 + cl
                                rs, cs = pk(ci)
                                rp_ = ci % 3
                                for t in range(2):
                                    pp = pprj.tile([128, NA], F32, tag="prj",
                                                   name="prj")
                                    dbcs = []
                                    for r in range(4):
                                        dbc = pdbc.tile([128, NA], F32,
                                                        tag="dbc", name="dbc")
                                        nc.tensor.matmul(
                                            dbc[:],
                                            cget('bsel4', rp_, r, t)[:],
                                            ap16[rs, cs], start=True,
                                            stop=True)
                                        dbcs.append(dbc)
                                    pr4 = tsb.tile([128, 4, NA], BF16,
                                                   tag="pr4", name="pr4")
                                    for r in range(4):
                                        vsl = VL[:, _VMAP[(r + 2 * t) % 4],
                                                 cl:cl + NA]
                                        if r in (1, 2):
                                            nc.vector.tensor_tensor(
                                                out=pr4[:, r, :],
                                                in0=dbcs[r][:],
                                                in1=vsl, op=OP.mult)
                                        else:
                                            db = tsb.tile([128, NA], BF16,
                                                          tag=f"dbb{r}",
                                                          name=f"dbb{r}")
                                            nc.scalar.activation(db[:],
                                                                 dbcs[r][:],
                                                                 AF.Copy)
                                            eng = nc.gpsimd if r == 3 \
                                                else nc.vector
                                            eng.tensor_tensor(
                                                out=pr4[:, r, :], in0=db[:],
                                                in1=vsl, op=OP.mult)
                                    ad1 = tsb.tile([128, 2, NA], BF16,
                                                   tag="ad1", name="ad1")
                                    nc.vector.tensor_tensor(
                                        out=ad1[:], in0=pr4[:, 0:2, :],
                                        in1=pr4[:, 2:4, :], op=OP.add)
                                    yo = tsb.tile([128, NA], BF16,
                                                  tag="yo", name="yo")
                                    nc.vector.tensor_tensor(
                                        out=yo[:], in0=ad1[:, 0, :],
                                        in1=ad1[:, 1, :], op=OP.add)
                                    nc.tensor.matmul(pp[:], cget('lp', t)[:],
                                                     yo[:], start=True,
                                                     stop=False)
                                    nc.tensor.matmul(pp[:], cget('da1', t)[:],
                                                     xb[:, t, cl:cl + NA],
                                                     start=False, stop=True)
                                    nc.scalar.activation(
                                        xres[t][:, 2 + cg:2 + cg + NA],
                                        pp[:], AF.Copy)

                # fallbacks so every variant still writes the full output
                if not do_A:
                    with tc.tile_pool(name="fxin", bufs=3) as fxi:
                        for ci in range(W // NA):
                            cg = ci * NA
                            for t in range(2):
                                xa = fxi.tile([128, NA], F32, tag=f"fx{t}",
                                              name=f"fx{t}")
                                nc.sync.dma_start(
                                    xa[:], xin[t][:, cg:cg + NA].bitcast(F32))
                                nc.vector.tensor_copy(
                                    xres[t][:, 2 + cg:2 + cg + NA], xa[:])
                if not do_B:
                    with tc.tile_pool(name="fob", bufs=3) as fob:
                        for ci in range(W // NA):
                            cg = ci * NA
                            for t in range(2):
                                xo = fob.tile([128, NA], F32, tag=f"fo{t}",
                                              name=f"fo{t}")
                                nc.vector.tensor_copy(
                                    xo[:], xres[t][:, 2 + cg:2 + cg + NA])
                                nc.sync.dma_start(xout[t][:, cg:cg + NA],
                                                  xo[:])
                # ============ SWEEP B (res block -> out), per half =========
                def sweepB(h):
                    H0 = h * W2
                    with tc.tile_pool(name="brow", bufs=1) as brp:
                        # y2 tiles: per (t,d): [128, 2(j), W2+2]
                        y2r = {}
                        for t in range(2):
                            for d in range(2):
                                y2r[(t, d)] = brp.tile(
                                    [128, 2, W2 + 2], BF16,
                                    tag=f"y2r{t}{d}", name=f"y2r{t}{d}")
                        # chunk ci -> rows 32*(ci%4)+0:4, col (ci//4)*NA
                        NB3 = ((nbh + 3) // 4) * NA
                        nsPk = brp.tile([128, NB3], F32, tag="nsPk",
                                        name="nsPk")
                        invp = brp.tile([128, NB3], F32, tag="invp",
                                        name="invp")

                        def pkb(ci):
                            rp_, cb = ci % 4, (ci // 4) * NA
                            return (slice(32 * rp_, 32 * rp_ + 4),
                                    slice(cb, cb + NA))

                        with (
                            tc.tile_pool(name="bst", bufs=3) as bst,
                            tc.tile_pool(name="by2", bufs=4,
                                         space="PSUM") as by2,
                            tc.tile_pool(name="bns", bufs=2,
                                         space="PSUM") as bns,
                            tc.tile_pool(name="bsq", bufs=3) as bsq,
                        ):
                            pend = {}
                            spre = {}

                            def b1_sdma(ci):
                                cl, C = hchunks[ci]
                                cg = H0 + cl
                                NW = C + 2
                                tiles = {}
                                for t in range(2):
                                    for d in range(2):
                                        S = bst.tile([128, NW + 1], BF16,
                                                     tag=f"S{t}{d}",
                                                     name=f"S{t}{d}")
                                        src = xres[t][64 * d:
                                                      64 * d + 64, :]
                                        nc.sync.dma_start(
                                            S[0:64, :],
                                            src[:, cg:cg + NW + 1])
                                        nc.sync.dma_start(
                                            S[64:128, :],
                                            src[:, cg + 1:cg + NW + 2])
                                        tiles[(t, d)] = S
                                spre[ci] = tiles

                            def b1_stage1(ci):
                                cl, C = hchunks[ci]
                                cg = H0 + cl
                                NW = C + 2
                                sqys = []
                                stiles = spre.pop(ci)
                                for t in range(2):
                                    for d in range(2):
                                        S = stiles[(t, d)]
                                        g = 2 * t + d
                                        yb = y2r[(t, d)]
                                        sqy = bsq.tile(
                                            [128, 2, NW], BF16,
                                            tag=f"sqy{t}{d}",
                                            name=f"sqy{t}{d}")
                                        for j in range(2):
                                            p = by2.tile([128, NW], F32,
                                                         tag="y2",
                                                         name="y2")
                                            nc.tensor.matmul(
                                                p[:],
                                                cget('l0a', g, j)[:],
                                                S[:, 0:NW],
                                                start=True, stop=False)
                                            nc.tensor.matmul(
                                                p[:],
                                                cget('l0b', g, j)[:],
                                                S[64:128, 1:NW + 1],
                                                start=False, stop=True)
                                            ysl = yb[:, j, cl:cl + NW]
                                            if j == 0:
                                                nc.scalar.activation(
                                                    ysl, p[:], AF.Copy)
                                                nc.vector.tensor_tensor(
                                                    out=sqy[:, j, :],
                                                    in0=ysl, in1=ysl,
                                                    op=OP.mult)
                                            else:
                                                nc.vector.tensor_copy(
                                                    ysl, p[:])
                                                if d == 0:
                                                    nc.scalar.activation(
                                                        sqy[:, j, :],
                                                        ysl, AF.Square)
                                                else:
                                                    nc.gpsimd.\
                                                        tensor_tensor(
                                                            out=sqy[:, j,
                                                                    :],
                                                            in0=ysl,
                                                            in1=ysl,
                                                            op=OP.mult)
                                        sqys.append((g, NW, sqy))
                                pend[ci] = sqys

                            def b1_stage2(ci):
                                NWc = hchunks[ci][1] + 2
                                nspc = bns.tile([4, NWc], F32, tag="nsp",
                                                name="nsp")
                                n = 0
                                for (g, NW, sqy) in pend.pop(ci):
                                    for j in range(2):
                                        nc.tensor.matmul(
                                            nspc[:, 0:NW],
                                            cget('onesg', g)[:],
                                            sqy[:, j, :],
                                            start=(n == 0),
                                            stop=(n == 7))
                                        n += 1
                                rs, cs = pkb(ci)
                                nc.vector.tensor_copy(
                                    nsPk[rs, cs.start:cs.start + NWc],
                                    nspc[:])

                            b1_sdma(0)
                            for s in range(nbh + 1):
                                if s + 1 < nbh:
                                    b1_sdma(s + 1)
                                if s < nbh:
                                    b1_stage1(s)
                                if s >= 1:
                                    b1_stage2(s - 1)

                        # ---- between: batched inv norms ----
                        snp = brp.tile([128, NB3], F32, tag="snp",
                                       name="snp")
                        nc.scalar.activation(snp[:], nsPk[:], AF.Sqrt,
                                             scale=1.0 / 256)
                        nc.vector.reciprocal_approx_fast(out=invp[:],
                                                         in_=snp[:])

                        with (
                            tc.tile_pool(name="bibc", bufs=4,
                                         space="PSUM") as bibc,
                            tc.tile_pool(name="bres", bufs=2,
                                         space="PSUM") as bres,
                            tc.tile_pool(name="bsb", bufs=2) as bsb,
                        ):
                            pend2 = {}

                            def b2_stage1(ci):
                                cl, C = hchunks[ci]
                                cg = H0 + cl
                                NW = C + 2
                                # per-chunk inv-norm rows, bf16 (base 0)
                                invc = bsb.tile([4, NW], BF16, tag="invc",
                                                name="invc")
                                rsb, csb = pkb(ci)
                                nc.vector.tensor_copy(
                                    invc[:],
                                    invp[rsb, csb.start:csb.start + NW])
                                ucs = []
                                for t in range(2):
                                    zcu = bsb.tile([128, 4, NW], BF16,
                                                   tag=f"zc{t}",
                                                   name=f"zc{t}")
                                    uc = bsb.tile([128, 4, NW], BF16,
                                                  tag=f"uc{t}",
                                                  name=f"uc{t}")
                                    for d in range(2):
                                        ib = bibc.tile([128, NW], F32,
                                                       tag="ibc", name="ibc")
                                        nc.tensor.matmul(
                                            ib[:], cget('ibsel4', t, d)[:],
                                            invc[:],
                                            start=True, stop=True)
                                        ibb = bsb.tile([128, NW], BF16,
                                                       tag=f"ibcb{t}{d}",
                                                       name=f"ibcb{t}{d}")
                                        if d == 0:
                                            nc.scalar.activation(
                                                ibb[:], ib[:], AF.Copy)
                                        else:
                                            nc.vector.tensor_copy(ibb[:],
                                                                  ib[:])
                                        for j in range(2):
                                            eng = nc.vector if j == 0 \
                                                else nc.gpsimd
                                            eng.tensor_tensor(
                                                out=zcu[:, 2 * d + j, :],
                                                in0=y2r[(t, d)][:, j,
                                                                cl:cl + NW],
                                                in1=ibb[:], op=OP.mult)
                                    nc.scalar.activation(uc[:], zcu[:],
                                                         AF.Silu)
                                    if cg == 0:
                                        nc.vector.memset(uc[:, :, 0:1], 0.0)
                                    if cg + C == W:
                                        nc.vector.memset(
                                            uc[:, :, NW - 1:NW], 0.0)
                                    ucs.append(uc)
                                pend2[ci] = ucs

                            def b2_stage2(ci):
                                cl, C = hchunks[ci]
                                cg = H0 + cl
                                ucs = pend2.pop(ci)
                                for t in range(2):
                                    uc = ucs[t]
                                    rp = bres.tile([128, C], F32, tag="rp",
                                                   name="rp")
                                    for col in range(2):
                                        kidx = 0
                                        for s in range(2):
                                            for tap in range(3):
                                                nc.tensor.matmul(
                                                    rp[64 * col:
                                                       64 * col + 64, :],
                                                    cget('l1', t,
                                                         s * 3 + tap,
                                                         col)[:],
                                                    uc[:, 2 * col + s,
                                                       tap:tap + C],
                                                    start=(kidx == 0),
                                                    stop=False)
                                                kidx += 1
                                    nc.tensor.matmul(
                                        rp[:], cget('da2', t)[:],
                                        xres[t][:, 2 + cg:2 + cg + C],
                                        start=False, stop=True,
                                        skip_group_check=True)
                                    xc = bsb.tile([128, C], F32,
                                                  tag=f"xc{t}",
                                                  name=f"xc{t}")
                                    nc.vector.tensor_scalar(
                                        out=xc[:], in0=rp[:], scalar1=CLIP,
                                        scalar2=-CLIP, op0=OP.min,
                                        op1=OP.max)
                                    nc.sync.dma_start(
                                        xout[t][:, cg:cg + C], xc[:])

                            for s in range(nbh + 1):
                                if s < nbh:
                                    b2_stage1(s)
                                if s >= 1:
                                    b2_stage2(s - 1)
                # NOTE: B's 1x3 conv needs the neighbor half's edge cols
                # of xres, so both A halves must complete before any B half.
                for h in range(2):
                    if do_A:
                        sweepA(h)
                for h in range(2):
                    if do_B:
                        sweepB(h)
    nc.compile()
    return nc


_BUILD_CACHE = {}


def _get_nc(W, reps=1, variant=''):
    key = (W, reps, variant)
    if key not in _BUILD_CACHE:
        _BUILD_CACHE[key] = build(W, reps=reps, variant=variant)
    return _BUILD_CACHE[key]


def make_in_map(inputs, b, W):
    x = np.asarray(inputs['x'], dtype=np.float32)
    cons = _host_prep(inputs, b)
    im = {'x': np.ascontiguousarray(x[b, :, 0, :].reshape(2, 128, W))}
    im.update(_pack_consts(cons))
    return im


def kernel(**inputs):
    x = np.asarray(inputs['x'], dtype=np.float32)
    B, C, H, W = x.shape
    nc = _get_nc(W)
    in_maps = [make_in_map(inputs, b, W) for b in range(B)]
    res = run_bass_kernel_spmd(nc, in_maps, list(range(B)))
    out = np.empty((B, C, H, W), np.float32)
    for b in range(B):
        out[b, :, 0, :] = np.asarray(res.results[b]['out'],
                                     dtype=np.float32).reshape(256, W)
    return out
